# revision 1
# baseline (speedup 1.0000x reference)
"""MatchLSTM attention kernel for 8 Trainium2 NeuronCores.

Reference computation (B=64, T=2048, D=512):
    G   = tanh(input_p@Wp.T + bp + input_q@Wq.T + bq + h_tm1@Wr.T + br)
    a   = softmax(G@w + match_b)            over T
    z   = sum_t a[:,t] * input_q[:,:,t]
    out = concat([input_p, z], -1)

Sharding: data-parallel over batch, 8 batches per core, weights replicated.

Per-core device pipeline (all matmul operands bf16, fp32 accumulation):
  - c^T[o,b] = (Wp.T;Wr.T;bias) matmuls against (input_p^T;h^T;ones)  [once]
  - X^T tiles [q,tok] via DMA-transpose; X natural tiles [tok,q] via DMA
  - G^T[o,tok] = Wq.T-chunk @ X^T-chunk (PE, fp32 PSUM)
  - tanh via ScalarE with per-partition bias c^T  -> bf16 SBUF
  - scores s[1,tok] = w-chunk.T @ tanhG (PE accum over o-chunks)
  - s transposed to columns via K=1 fp16 matmuls; exp(s+match_b) on ScalarE
    -> bf16, with sumexp accumulated for free via activation accum_out
  - z[1,512] = sum_j esc_j.T @ Xnat_j (PE, fp32 PSUM accumulation)
  - z scaled by 1/sumexp (VectorE), DMA out.  Softmax max-subtraction is
    skipped: |s| <= sum|w| + 1 < 25, exp stays well inside fp32 range.
"""

import sys

if "/opt/trn_rl_repo" not in sys.path:
    sys.path.insert(0, "/opt/trn_rl_repo")

import numpy as np
import ml_dtypes

N_CORES = 8
B, T, D = 64, 2048, 512
PB = B // N_CORES          # batches per core
KC = D // 128              # 4 contraction chunks of 128
NTT = T // 512             # 4 token tiles of 512
NJ = T // 128              # 16 token chunks of 128
CROWS = 2 * D + 128        # cw/cx rows: Wp.T, Wr.T, bias row + zero pad

BF16 = ml_dtypes.bfloat16

_CACHE: dict = {}


def _build_program():
    import concourse.bacc as bacc
    import concourse.tile as tile
    import concourse.mybir as mybir
    from concourse.bass import MemorySpace

    dt = mybir.dt
    F32 = dt.float32
    BF = dt.bfloat16
    AF = mybir.ActivationFunctionType

    nc = bacc.Bacc(
        "TRN2", target_bir_lowering=False, debug=False, num_devices=N_CORES
    )

    xq_d = nc.dram_tensor("xq", [PB, T, D], BF, kind="ExternalInput")
    wq_d = nc.dram_tensor("wqt", [D, D], BF, kind="ExternalInput")      # Wq.T [q,o]
    cw_d = nc.dram_tensor("cw", [CROWS, D], BF, kind="ExternalInput")   # [Wp.T;Wr.T;bias;0]
    cx_d = nc.dram_tensor("cx", [CROWS, PB], BF, kind="ExternalInput")  # [ip.T;h.T;1;0]
    wcol_d = nc.dram_tensor("wcol", [D, 1], BF, kind="ExternalInput")
    mb_d = nc.dram_tensor("mb", [128, 1], F32, kind="ExternalInput")    # match_b bcast
    z_d = nc.dram_tensor("z", [1, PB * D], F32, kind="ExternalOutput")

    NKC = CROWS // 128  # 9 contraction chunks for the c matmuls

    F16 = dt.float16

    with tile.TileContext(nc) as tc:
        with (
            tc.tile_pool(name="consts", bufs=1) as consts,
            tc.tile_pool(name="xT_p", bufs=3) as xT_pool,
            tc.tile_pool(name="xnat_p", bufs=3) as xnat_pool,
            tc.tile_pool(name="tanh_p", bufs=8) as tanh_pool,
            tc.tile_pool(name="srow_p", bufs=3) as srow_pool,
            tc.tile_pool(name="esc_p", bufs=3) as esc_pool,
            tc.tile_pool(name="small_p", bufs=2) as small_pool,
            tc.tile_pool(name="zout_p", bufs=1) as zout_pool,
            tc.tile_pool(name="pG", bufs=2, space=MemorySpace.PSUM) as pG,
            tc.tile_pool(name="pS", bufs=2, space=MemorySpace.PSUM) as pS,
            tc.tile_pool(name="pZ", bufs=1, space=MemorySpace.PSUM) as pZ,
            tc.tile_pool(name="pM", bufs=1, space=MemorySpace.PSUM) as pM,
        ):
            # ---- constants (DMA order = criticality order) -----------------
            cw_s = consts.tile([128, NKC, D], BF, tag="cw", name="cw_s")
            nc.sync.dma_start(out=cw_s, in_=cw_d.rearrange("(c p) o -> p c o", p=128))
            cx_s = consts.tile([128, NKC, PB], BF, tag="cx", name="cx_s")
            nc.sync.dma_start(out=cx_s, in_=cx_d.rearrange("(c p) b -> p c b", p=128))
            wq_s = consts.tile([128, KC, D], BF, tag="wq", name="wq_s")
            nc.sync.dma_start(out=wq_s, in_=wq_d.rearrange("(c p) o -> p c o", p=128))
            wcol_s = consts.tile([128, KC, 1], BF, tag="wcol", name="wcol_s")
            nc.sync.dma_start(out=wcol_s, in_=wcol_d.rearrange("(c p) o -> p c o", p=128))
            mb_s = consts.tile([128, 1], F32, tag="mb", name="mb_s")
            nc.sync.dma_start(out=mb_s, in_=mb_d[:, :])
            ones128 = consts.tile([128, 1], F32, tag="ones128", name="ones128")
            nc.vector.memset(ones128, 1.0)
            ones_f16 = consts.tile([1, 1], F16, tag="ones_f16", name="ones_f16")
            nc.vector.memset(ones_f16, 1.0)
            # warm the ACT table set (tanh/exp share one set) off the critical path
            dummy_s = consts.tile([1, 1], F32, tag="dummy", name="dummy_s")
            nc.scalar.activation(
                out=dummy_s, in_=ones_f16, func=AF.Tanh, bias=0.0, scale=1.0
            )

            # ---- c^T[o, b] for all batches (once) --------------------------
            c_ps = pM.tile([128, KC, PB], F32, tag="misc", name="c_ps")
            for oc in range(KC):
                for k in range(NKC):
                    nc.tensor.matmul(
                        c_ps[:, oc, :],
                        cw_s[:, k, oc * 128 : (oc + 1) * 128],
                        cx_s[:, k, :],
                        start=(k == 0),
                        stop=(k == NKC - 1),
                    )
            cT_s = consts.tile([128, KC, PB], F32, tag="cT", name="cT_s")
            nc.vector.tensor_copy(out=cT_s, in_=c_ps)

            zout_s = zout_pool.tile([1, PB, D], F32, tag="zout", name="zout_s")

            # ---- per-batch pipeline ---------------------------------------
            for b in range(PB):
                xT = xT_pool.tile([128, KC, T], BF, tag="xT", name="xT")
                # batch 0 is latency-critical: land the first half-T of each
                # q-chunk sooner by splitting the transposes.
                nh = 2 if b == 0 else 1
                for h in range(nh):
                    for qc in range(KC):
                        nc.sync.dma_start(
                            out=xT[:, qc, h * (T // nh) : (h + 1) * (T // nh)],
                            in_=xq_d[
                                b,
                                h * (T // nh) : (h + 1) * (T // nh),
                                qc * 128 : (qc + 1) * 128,
                            ],
                            transpose=True,
                        )
                xnat = xnat_pool.tile([128, NJ, D], BF, tag="xnat", name="xnat")
                nc.sync.dma_start(
                    out=xnat, in_=xq_d[b].rearrange("(i p) q -> p i q", p=128)
                )

                s_cat = srow_pool.tile([1, T], F16, tag="scat", name="s_cat")
                esc = esc_pool.tile([128, NJ], BF, tag="esc", name="esc")
                pesum = small_pool.tile([128, 2], F32, tag="pesum", name="pesum")
                z_ps = pZ.tile([1, D], F32, tag="z", name="z_ps")
                # token tiles processed in pairs sharing one [128,1024] PSUM
                # G tile (2 banks): same Wq chunk stays loaded across the pair
                # and tanh runs once per 1024 tokens.
                for tp in range(NTT // 2):
                    sT_ps = pM.tile([128, NJ // 2], F32, tag="misc", name="sT_ps")
                    sc_pair = [
                        pS.tile([1, 512], F32, tag="s", name="sc_ps")
                        for _ in range(2)
                    ]
                    for oc in range(KC):
                        g_ps = pG.tile([128, 1024], F32, tag="g", name="g_ps")
                        for qc in range(KC):
                            for i in range(2):
                                tt = tp * 2 + i
                                nc.tensor.matmul(
                                    g_ps[:, i * 512 : (i + 1) * 512],
                                    wq_s[:, qc, oc * 128 : (oc + 1) * 128],
                                    xT[:, qc, tt * 512 : (tt + 1) * 512],
                                    start=(qc == 0),
                                    stop=(qc == KC - 1),
                                )
                        th = tanh_pool.tile([128, 1024], BF, tag="th", name="th")
                        nc.scalar.activation(
                            out=th,
                            in_=g_ps,
                            func=AF.Tanh,
                            bias=cT_s[:, oc, b : b + 1],
                            scale=1.0,
                        )
                        for i in range(2):
                            nc.tensor.matmul(
                                sc_pair[i],
                                wcol_s[:, oc, :],
                                th[:, i * 512 : (i + 1) * 512],
                                start=(oc == 0),
                                stop=(oc == KC - 1),
                            )
                    for i in range(2):
                        tt = tp * 2 + i
                        nc.vector.tensor_copy(
                            out=s_cat[:, tt * 512 : (tt + 1) * 512], in_=sc_pair[i]
                        )
                        # transpose scores into columns (K=1 fp16 matmuls)
                        for jj in range(4):
                            j = tt * 4 + jj
                            nc.tensor.matmul(
                                sT_ps[:, j - tp * 8 : j - tp * 8 + 1],
                                s_cat[:, j * 128 : (j + 1) * 128],
                                ones_f16,
                                start=True,
                                stop=True,
                            )
                    # exp + its half of the z accumulation start mid-batch
                    nc.scalar.activation(
                        out=esc[:, tp * 8 : (tp + 1) * 8],
                        in_=sT_ps,
                        func=AF.Exp,
                        bias=mb_s,
                        scale=1.0,
                        accum_out=pesum[:, tp : tp + 1],
                    )
                    for j in range(tp * 8, (tp + 1) * 8):
                        nc.tensor.matmul(
                            z_ps,
                            esc[:, j : j + 1],
                            xnat[:, j, :],
                            start=(j == 0),
                            stop=(j == NJ - 1),
                        )

                se_ps = pM.tile([1, 2], F32, tag="misc", name="se_ps")
                nc.tensor.matmul(se_ps, ones128, pesum, start=True, stop=True)
                se_sb = small_pool.tile([1, 2], F32, tag="sesb", name="se_sb")
                nc.vector.tensor_copy(out=se_sb, in_=se_ps)
                se_tot = small_pool.tile([1, 1], F32, tag="setot", name="se_tot")
                nc.vector.tensor_add(se_tot, se_sb[:, 0:1], se_sb[:, 1:2])
                rse_s = small_pool.tile([1, 1], F32, tag="rse", name="rse_s")
                nc.vector.reciprocal(out=rse_s, in_=se_tot)

                nc.vector.tensor_scalar_mul(
                    out=zout_s[:, b, :], in0=z_ps, scalar1=rse_s
                )

            nc.sync.dma_start(out=z_d[:, :], in_=zout_s)

    nc.compile()
    return nc


def _get_program():
    if "nc" not in _CACHE:
        _CACHE["nc"] = _build_program()
    return _CACHE["nc"]


def kernel(**inputs) -> np.ndarray:
    from concourse import bass_utils

    inp = {k: np.asarray(v) for k, v in inputs.items()}
    input_p = inp["input_p"].astype(np.float32)
    input_q = inp["input_q"].astype(np.float32)
    h_tm1 = inp["h_tm1"].astype(np.float32)
    Wp, Wq, Wr = inp["Wp"], inp["Wq"], inp["Wr"]
    bp, bq, br = inp["bp"], inp["bq"], inp["br"]
    w = inp["w"]
    mb = float(np.asarray(inp["match_b"]).reshape(-1)[0])

    # shared (weight) tensors
    wqt = np.ascontiguousarray(Wq.T).astype(BF16)
    cw = np.zeros((CROWS, D), dtype=BF16)
    cw[:D] = Wp.T.astype(BF16)
    cw[D : 2 * D] = Wr.T.astype(BF16)
    cw[2 * D] = (bp.astype(np.float32) + bq + br).astype(BF16)
    wcol = np.ascontiguousarray(w.reshape(D, 1)).astype(BF16)
    mb_arr = np.full((128, 1), mb, dtype=np.float32)

    nc = _get_program()

    in_maps = []
    for c in range(N_CORES):
        s = slice(c * PB, (c + 1) * PB)
        cx = np.zeros((CROWS, PB), dtype=BF16)
        cx[:D] = input_p[s].T.astype(BF16)
        cx[D : 2 * D] = h_tm1[s].T.astype(BF16)
        cx[2 * D] = 1.0
        in_maps.append(
            {
                "xq": np.ascontiguousarray(input_q[s]).astype(BF16),
                "wqt": wqt,
                "cw": cw,
                "cx": cx,
                "wcol": wcol,
                "mb": mb_arr,
            }
        )

    res = bass_utils.run_bass_kernel_spmd(
        nc, in_maps, core_ids=list(range(N_CORES))
    )
    z = np.concatenate(
        [
            np.asarray(res.results[c]["z"], dtype=np.float32).reshape(PB, D)
            for c in range(N_CORES)
        ],
        axis=0,
    )
    return np.concatenate([input_p, z], axis=1)



# revision 8
# speedup vs baseline: 2.3687x; 2.3687x over previous
"""MatchLSTM attention kernel for 8 Trainium2 NeuronCores.

Reference computation (B=64, T=2048, D=512):
    G   = tanh(input_p@Wp.T + bp + input_q@Wq.T + bq + h_tm1@Wr.T + br)
    a   = softmax(G@w + match_b)            over T
    z   = sum_t a[:,t] * input_q[:,:,t]
    out = concat([input_p, z], -1)

Sharding: data-parallel over batch, 8 batches per core, weights replicated.

Per-core pipeline (ACT-bound: tanh of G is the irreducible cost):
  - c[b,o] = input_p@Wp.T + h@Wr.T + (bp+bq+br) computed on HOST in fp32,
    uploaded as the per-partition tanh bias.  match_b dropped (softmax
    shift-invariant).
  - EVERY PE matmul uses fp8e4m3 DoubleRow (contraction 256 per call,
    0.5 cycles/out-col).  Mixing DoubleRow with normal-mode matmuls was
    observed to corrupt PE results non-deterministically on hw, so the
    kernel keeps the PE in one mode throughout:
      * G^T[o,t] tiles [128,1024]: stationary Wq^T chunk, moving X^T.
      * scores: tanh-pair tile [128o,2,128t] STATIONARY, w pair-column
        moving -> sT column [128t,1] directly transposed; a second pass
        with the fp8 residual of w cancels w's quantization error.
      * z: xnat chunk-pair [128t,2,128q] STATIONARY, esc column pair
        moving -> z[128q,1].
  - tanh fused with bias on ScalarE -> fp8 SBUF, oc-pairs sharing one
    [128,2,1024] tile; exp once per batch ([128,16]) -> fp8 esc with
    per-partition sumexp via accum_out.
  - Raw z and pesum are DMA'd out; the host performs the 1/sumexp scale
    (bit-equivalent fp32 divide, removes all non-DoubleRow PE work).
  - Score matmuls are emitted with a one-tile lag and batch tails with a
    two-tile lag so the in-order PE queue never blocks the ACT engine.
"""

import sys

if "/opt/trn_rl_repo" not in sys.path:
    sys.path.insert(0, "/opt/trn_rl_repo")

import numpy as np
import ml_dtypes

N_CORES = 8
B, T, D = 64, 2048, 512
PB = B // N_CORES          # batches per core
NJ = T // 128              # 16 token chunks of 128 (esc/xnat granularity)

BF16 = ml_dtypes.bfloat16
FP8 = ml_dtypes.float8_e4m3

_CACHE: dict = {}


def _build_program():
    import concourse.bacc as bacc
    import concourse.tile as tile
    import concourse.mybir as mybir
    from concourse.bass import MemorySpace

    dt = mybir.dt
    F32 = dt.float32
    F8 = dt.float8e4
    AF = mybir.ActivationFunctionType
    DR = mybir.MatmulPerfMode.DoubleRow

    nc = bacc.Bacc(
        "TRN2", target_bir_lowering=False, debug=False, num_devices=N_CORES
    )

    # dram inputs (host-prepared layouts, all DMAs are contiguous copies)
    xqT_d = nc.dram_tensor("xqT", [PB, 128, 2, 2, T], F8, kind="ExternalInput")
    xnat_d = nc.dram_tensor("xnat", [PB, 128, NJ, 512], F8, kind="ExternalInput")
    wq_d = nc.dram_tensor("wqt", [128, 2, 2, D], F8, kind="ExternalInput")
    ct_d = nc.dram_tensor("ct", [128, 4, PB], F32, kind="ExternalInput")
    # w split into fp8 main + fp8 residual, laid out as [p, ocpair, u, 16]
    # (padded so the DR pair-dim stride is 16 elements, an ISA requirement)
    wcol_d = nc.dram_tensor("wcol", [128, 2, 2, 32], F8, kind="ExternalInput")
    z_d = nc.dram_tensor("z", [PB, 128, 4], F32, kind="ExternalOutput")
    p_d = nc.dram_tensor("pe", [PB, 128, 1], F32, kind="ExternalOutput")

    with tile.TileContext(nc) as tc:
        with (
            tc.tile_pool(name="consts", bufs=1) as consts,
            tc.tile_pool(name="xT_p", bufs=3) as xT_pool,
            tc.tile_pool(name="xnat_p", bufs=3) as xnat_pool,
            tc.tile_pool(name="th_p", bufs=3) as th_pool,
            tc.tile_pool(name="esc_p", bufs=2) as esc_pool,
            tc.tile_pool(name="small_p", bufs=2) as small_pool,
            tc.tile_pool(name="pG", bufs=2, space=MemorySpace.PSUM) as pG,
            tc.tile_pool(name="pST", bufs=1, space=MemorySpace.PSUM) as pST,
            tc.tile_pool(name="pZ", bufs=1, space=MemorySpace.PSUM) as pZ,
        ):
            # ---- constants (DMA order = criticality order) -----------------
            wq_s = consts.tile([128, 2, 2, D], F8, tag="wq", name="wq_s")
            nc.sync.dma_start(out=wq_s, in_=wq_d[:, :, :, :])
            cT_s = consts.tile([128, 4, PB], F32, tag="cT", name="cT_s")
            nc.sync.dma_start(out=cT_s, in_=ct_d[:, :, :])
            wcol_s = consts.tile([128, 2, 2, 32], F8, tag="wcol", name="wcol_s")
            nc.sync.dma_start(out=wcol_s, in_=wcol_d[:, :, :, :])

            # per-batch state captured across the lagged emission stream
            st: dict = {}

            def batch_start(b):
                xT = xT_pool.tile([128, 2, 2, T], F8, tag="xT", name="xT")
                # split the transfer so the first token-half lands sooner
                for h in range(2):
                    nc.sync.dma_start(
                        out=xT[:, :, :, h * 1024 : (h + 1) * 1024],
                        in_=xqT_d[b, :, :, :, h * 1024 : (h + 1) * 1024],
                    )
                xnat = xnat_pool.tile([128, NJ, 512], F8, tag="xnat", name="xnat")
                nc.sync.dma_start(out=xnat, in_=xnat_d[b])
                esc = esc_pool.tile([128, NJ // 2, 2, 16], F8, tag="esc", name="esc")
                nc.vector.memset(esc, 0.0)  # pad cols must be 0 for the z matmul
                s_sb = small_pool.tile([128, NJ // 2, 2], F32, tag="ssb", name="s_sb")
                pesum = small_pool.tile([128, 1], F32, tag="pesum", name="pesum")
                st[b] = dict(xT=xT, xnat=xnat, s_sb=s_sb, esc=esc, pesum=pesum)

            def emit_g(b, h, oc):
                xT = st[b]["xT"]
                g_ps = pG.tile([128, 1024], F32, tag="g", name="g_ps")
                for g2 in range(2):
                    for i in range(2):
                        t0 = h * 1024 + i * 512
                        nc.tensor.matmul(
                            g_ps[:, i * 512 : (i + 1) * 512],
                            wq_s[:, g2, :, oc * 128 : (oc + 1) * 128],
                            xT[:, g2, :, t0 : t0 + 512],
                            start=(g2 == 0),
                            stop=(g2 == 1),
                            perf_mode=DR,
                        )
                return g_ps

            def emit_tanh(b, h, oc, g_ps):
                # oc-pairs share one [128, 2, 1024] fp8 tile (DR stationary)
                if oc % 2 == 0:
                    st[b]["th2"] = th_pool.tile(
                        [128, 2, 1024], F8, tag="th", name="th2"
                    )
                th2 = st[b]["th2"]
                nc.scalar.activation(
                    out=th2[:, oc % 2, :],
                    in_=g_ps,
                    func=AF.Tanh,
                    bias=cT_s[:, oc, b : b + 1],
                    scale=1.0,
                )
                return th2

            def emit_scores(b, h, oc, th2):
                # called after the odd-oc tanh of pair ocp = oc // 2.
                # Each score column owns a full 512B PSUM quadrant: a
                # DoubleRow start=True wipes the whole quadrant, so open
                # groups must never share one.
                ocp = oc // 2
                if ocp == 0 and "sT" not in st[b]:
                    st[b]["sT"] = pST.tile([128, 8, 128], F32, tag="st", name="sT_ps")
                sT_ps = st[b]["sT"]
                for jj in range(8):
                    for r in range(2):  # w main + residual
                        nc.tensor.matmul(
                            sT_ps[:, jj, 0:16],
                            th2[:, :, jj * 128 : (jj + 1) * 128],
                            wcol_s[:, ocp, :, 16 * r : 16 * r + 16],
                            start=(ocp == 0 and r == 0),
                            stop=(ocp == 1 and r == 1),
                            perf_mode=DR,
                        )
                if ocp == 1:
                    # evacuate this half's 8 scores so the quadrants can be
                    # reused by the other half
                    s_sb = st[b]["s_sb"]
                    j0 = h * 8
                    nc.vector.tensor_copy(
                        out=s_sb.rearrange("p m u -> p (m u)")[:, j0 : j0 + 8],
                        in_=sT_ps[:, :, 0],
                    )
                    st[b].pop("sT")

            def batch_tail(b):
                s = st.pop(b)
                esc, pesum, xnat = s["esc"], s["pesum"], s["xnat"]
                # exp of all 16 score columns; per-partition sumexp for free
                nc.scalar.activation(
                    out=esc[:, :, :, 0],
                    in_=s["s_sb"],
                    func=AF.Exp,
                    bias=0.0,
                    scale=1.0,
                    accum_out=pesum,
                )
                # z[q] = sum_t esc_t * X[t, q]  (xnat chunk-pair stationary)
                z_ps = pZ.tile([128, 4, 128], F32, tag="z", name="z_ps")
                for qc in range(4):
                    for m in range(NJ // 2):
                        nc.tensor.matmul(
                            z_ps[:, qc, 0:16],
                            xnat[:, 2 * m : 2 * m + 2, qc * 128 : (qc + 1) * 128],
                            esc[:, m, :, :],
                            start=(m == 0),
                            stop=(m == NJ // 2 - 1),
                            perf_mode=DR,
                        )
                zr = small_pool.tile([128, 4], F32, tag="zr", name="zr")
                nc.vector.tensor_copy(out=zr, in_=z_ps[:, :, 0])

                nc.sync.dma_start(out=z_d[b], in_=zr)
                nc.sync.dma_start(out=p_d[b], in_=pesum)

            # ---- lagged emission stream -----------------------------------
            tiles = [(b, h, oc) for b in range(PB) for h in range(2) for oc in range(4)]
            NTILES = len(tiles)
            pending: dict = {}
            for idx in range(NTILES + 2):
                if idx < NTILES:
                    b, h, oc = tiles[idx]
                    if h == 0 and oc == 0:
                        batch_start(b)
                    g_ps = emit_g(b, h, oc)
                # batch tail with two-tile lag (keeps ACT fed at boundaries)
                if idx >= 2:
                    pb_, ph_, poc_ = tiles[idx - 2]
                    if ph_ == 1 and poc_ == 3:
                        batch_tail(pb_)
                # scores with one-tile lag, after each odd-oc tanh
                if 1 <= idx <= NTILES:
                    pb_, ph_, poc_ = tiles[idx - 1]
                    if poc_ % 2 == 1:
                        emit_scores(pb_, ph_, poc_, pending.pop(idx - 1))
                if idx < NTILES:
                    pending[idx] = emit_tanh(b, h, oc, g_ps)

    nc.compile()
    return nc


def _get_program():
    if "nc" not in _CACHE:
        _CACHE["nc"] = _build_program()
    return _CACHE["nc"]


def kernel(**inputs) -> np.ndarray:
    from concourse import bass_utils

    inp = {k: np.asarray(v) for k, v in inputs.items()}
    input_p = inp["input_p"].astype(np.float32)
    input_q = inp["input_q"].astype(np.float32)
    h_tm1 = inp["h_tm1"].astype(np.float32)
    Wp, Wq, Wr = inp["Wp"], inp["Wq"], inp["Wr"]
    bp, bq, br = inp["bp"], inp["bq"], inp["br"]
    w = np.asarray(inp["w"], dtype=np.float32)
    # match_b is a constant shift of the pre-softmax scores: softmax-invariant.

    # shared (weight) tensors
    wqt = np.ascontiguousarray(
        Wq.T.reshape(2, 2, 128, D).transpose(2, 0, 1, 3)
    ).astype(FP8)
    # w as fp8 main + fp8 residual (second DR pass cancels quantization)
    w8 = w.astype(FP8)
    wres = (w - w8.astype(np.float32)).astype(FP8)
    wcol = np.zeros((128, 2, 2, 32), dtype=FP8)
    wcol[:, :, :, 0] = w8.reshape(2, 2, 128).transpose(2, 0, 1)
    wcol[:, :, :, 16] = wres.reshape(2, 2, 128).transpose(2, 0, 1)
    # c[b,o] = input_p@Wp.T + h@Wr.T + (bp+bq+br), fp32 on host
    c = (
        input_p @ Wp.T.astype(np.float32)
        + h_tm1 @ Wr.T.astype(np.float32)
        + (bp + bq + br).astype(np.float32)
    )

    nc = _get_program()

    in_maps = []
    for cix in range(N_CORES):
        s = slice(cix * PB, (cix + 1) * PB)
        xq = input_q[s]  # (PB, T, D)
        xqT = np.ascontiguousarray(
            xq.transpose(0, 2, 1).reshape(PB, 2, 2, 128, T).transpose(0, 3, 1, 2, 4)
        ).astype(FP8)
        xnat = np.ascontiguousarray(
            xq.reshape(PB, NJ, 128, D).transpose(0, 2, 1, 3)
        ).astype(FP8)
        ct = np.ascontiguousarray(
            c[s].T.reshape(4, 128, PB).transpose(1, 0, 2)
        ).astype(np.float32)
        in_maps.append(
            {"xqT": xqT, "xnat": xnat, "wqt": wqt, "ct": ct, "wcol": wcol}
        )

    res = bass_utils.run_bass_kernel_spmd(
        nc, in_maps, core_ids=list(range(N_CORES))
    )
    zs = []
    for cix in range(N_CORES):
        zraw = np.asarray(res.results[cix]["z"], dtype=np.float32)   # [PB,128,4]
        pes = np.asarray(res.results[cix]["pe"], dtype=np.float32)   # [PB,128,1]
        S = pes[:, :, 0].sum(axis=1)                                  # [PB]
        zs.append(
            (zraw.transpose(0, 2, 1).reshape(PB, D) / S[:, None]).astype(np.float32)
        )
    z = np.concatenate(zs, axis=0)
    return np.concatenate([input_p, z], axis=1)
